# revision 2
# baseline (speedup 1.0000x reference)
"""2-layer GCN (GCNConv -> ReLU -> GCNConv) on 8 Trainium2 NeuronCores.

Math:  out = Ahat @ relu(Ahat @ X @ W1 + b1) @ W2 + b2,
       Ahat = D^-1/2 (A + I) D^-1/2  (in-degree from dst, self-loops added).

Strategy (all hardcoded for N=100000, E=3200000, 512->16->40, 8 cores):
  * Factor the symmetric norm: pre-scale table rows by dinv, post-scale
    aggregated rows by dinv, self-loop handled algebraically (+table[own row]).
  * Commute W2 past aggregation: both layers aggregate 16-wide tables.
  * Nodes sharded 8-way, degree-profile interleaved so every core has an
    identical compile-time slot schedule (SPMD: one instruction stream).
  * Per layer: matmul/epilogue -> shard table -> AllGather -> dma_gather
    (SWDGE embedding-gather ucode, 4 queues, int16 indices mid-anchored per
    half-table segment, 64B payload on a 256B-stride padded table) into
    per-node slot grids -> strided DVE reduce -> epilogue.
"""

import sys

sys.path.insert(0, "/opt/trn_rl_repo")

import inspect
import textwrap

import numpy as np

import concourse.bacc as bacc
import concourse.bass as bass
import concourse.mybir as mybir
import concourse.tile as tile
from concourse._compat import cdiv
from concourse.bass_utils import run_bass_kernel_spmd
from concourse.masks import make_identity

F32 = mybir.dt.float32
I16 = mybir.dt.int16

N_CORES = 8
P = 128


# dma_gather with the elem_size%256 assert relaxed: the SWDGE ucode supports an
# arbitrary payload per index; only the row STRIDE must be a multiple of 256B.
def _make_dma_gather_raw():
    s = textwrap.dedent(inspect.getsource(bass.BassGpSimd.dma_gather))
    old = """    assert (
        elem_size_bytes > 0 and elem_size_bytes % 256 == 0
    )  # transpose restriction"""
    new = """    assert elem_size_bytes > 0
    if transpose:
        assert elem_size_bytes % 256 == 0"""
    assert old in s
    s = s.replace(old, new)
    g = dict(bass.__dict__)
    exec(compile(s, "<dma_gather_raw>", "exec"), g)
    return g["dma_gather"]


DMA_GATHER_RAW = _make_dma_gather_raw()


class Config:
    def __init__(self, n_nodes, f_in, hidden, f_out, percore, lmax_call=24):
        assert percore % P == 0
        self.n_nodes = n_nodes
        self.f_in = f_in
        self.hidden = hidden
        self.f_out = f_out
        self.percore = percore
        self.tiles = percore // P
        self.nrows = percore * N_CORES
        self.row_pad = 64           # table row stride in f32 elements (256B)
        self.lmax_call = lmax_call  # max slot-depth per dma_gather call
        # Piece-major table layout for pipelined AllGathers:
        #   piece a = locals [0, 4352)      -> table rows [0, 34816)
        #   piece b = locals [4352, 8192)   -> table rows [34816, 65536)
        #   piece c = locals [8192, 12544)  -> table rows [65536, 100352)
        # Windows: w0 = rows [0, 65536) (tensor tab_a, ready after AG a+b),
        #          w1 = rows [34816, 100352) (tensor tab_b = copy of the
        #          piece-b band + AG c). Piece-b rows are reachable from both
        #          windows -> flex balancing of the two slot grids.
        assert percore == 12544 and N_CORES == 8
        self.piece_lo = [0, 4352, 8192]       # local-row piece boundaries
        self.piece_hi = [4352, 8192, 12544]
        self.piece_base = [0, 34816, 65536]   # table-row base of each piece
        self.win_lo = [0, 34816]
        self.win_hi = [65536, self.nrows]
        self.seg_anchor = [32768, 34816 + 32768]
        # reserved guaranteed-zero local rows (never assigned a node):
        #   local 8191 (last of piece b)  -> table row 65535 (w0 pad, core 7)
        #   local 12543 (last of piece c) -> table row 100351 (w1 pad, core 7)
        self.reserved_local = 8191
        self.pad_row = [65535, 100351]
        for g in (0, 1):
            rel = self.pad_row[g] - self.seg_anchor[g]
            assert 0 < rel <= 32767, rel
            assert self.win_lo[g] - self.seg_anchor[g] >= -32768
            assert self.win_hi[g] - 1 - self.seg_anchor[g] <= 32767

    def tabrow_of_local(self, core, local):
        """Piece-major table row for (core, local) — vectorized."""
        import numpy as _np
        local = _np.asarray(local)
        p = _np.searchsorted(_np.array(self.piece_hi), local, side="right")
        size = _np.array([hi - lo for lo, hi in zip(self.piece_lo, self.piece_hi)])
        return (
            _np.array(self.piece_base)[p]
            + core * size[p]
            + (local - _np.array(self.piece_lo)[p])
        )


def host_prep(x, edge_index, cfg: Config, interleave=True):
    """Graph partitioning: relabel nodes, build per-core slot grids + schedule."""
    n = cfg.n_nodes
    src_o = np.asarray(edge_index[0], dtype=np.int64)
    dst_o = np.asarray(edge_index[1], dtype=np.int64)
    deg = np.bincount(dst_o, minlength=n).astype(np.int64)  # in-degree, no self loop

    # pass 1: global degree sort -> core assignment (rank % 8) so all cores
    # see nearly identical degree profiles.
    rank_of = np.empty(n, dtype=np.int64)
    order = np.argsort(deg, kind="stable")
    rank_of[order] = np.arange(n)
    if interleave:
        core_of = rank_of % N_CORES
    else:
        core_of = rank_of // (n // N_CORES)

    W0_HI, W1_LO = cfg.win_hi[0], cfg.win_lo[1]
    n_pc = n // N_CORES
    # available local slots (reserved zero row 8191 skipped)
    loc_slots = np.concatenate(
        [np.arange(cfg.reserved_local), np.arange(cfg.reserved_local + 1, cfg.percore)]
    )[:n_pc]

    # two passes: order within core by (deg, forced-w0 in-degree); the forced
    # class of a src depends on its local slot, so iterate once.
    local_of = np.empty(n, dtype=np.int64)
    for c in range(N_CORES):
        nodes = np.where(core_of == c)[0]
        nodes = nodes[np.argsort(deg[nodes], kind="stable")]
        local_of[nodes] = loc_slots
    for _ in range(2):
        tabrow = cfg.tabrow_of_local(core_of, local_of)
        src_t = tabrow[src_o]
        c1 = np.bincount(dst_o[src_t < W1_LO], minlength=n)  # forced w0
        local_of = np.empty(n, dtype=np.int64)
        for c in range(N_CORES):
            nodes = np.where(core_of == c)[0]
            o = np.lexsort((c1[nodes], deg[nodes]))
            nodes = nodes[o]
            local_of[nodes] = loc_slots

    row_of = core_of * cfg.percore + local_of
    tabrow = cfg.tabrow_of_local(core_of, local_of)
    src_r = tabrow[src_o]          # table rows (piece-major) for gather idx
    dst_r = row_of[dst_o]          # output rows (core-major)

    dst_core = dst_r // cfg.percore
    dst_local = dst_r % cfg.percore
    dst_tile = dst_local // P

    # forced / flex classification per edge
    forced0 = src_r < W1_LO
    forced2 = src_r >= W0_HI
    flex = ~(forced0 | forced2)

    # per-(core,tile,partition) forced counts + flex counts
    key_node = (dst_core * cfg.tiles + dst_tile) * P + (dst_local % P)
    nslots = N_CORES * cfg.tiles * P
    C1 = np.bincount(key_node[forced0], minlength=nslots).reshape(N_CORES, cfg.tiles, P)
    C2 = np.bincount(key_node[forced2], minlength=nslots).reshape(N_CORES, cfg.tiles, P)
    FL = np.bincount(key_node[flex], minlength=nslots).reshape(N_CORES, cfg.tiles, P)
    DEG = C1 + C2 + FL

    # choose per-tile (L0, L1) minimizing L0+L1 s.t. per-node loads fit
    Lassign = np.zeros((cfg.tiles, 2), dtype=np.int64)
    take1 = np.zeros((N_CORES, cfg.tiles, P), dtype=np.int64)  # flex sent to w0
    for t in range(cfg.tiles):
        c1t, c2t, degt = C1[:, t, :], C2[:, t, :], DEG[:, t, :]
        lo = int(c1t.max())
        hi = int((c1t + FL[:, t, :]).max())
        best = None
        for L0 in range(lo, hi + 1):
            need2 = int(np.maximum(c2t, degt - L0).max())
            if best is None or L0 + need2 < best[0] + best[1]:
                best = (L0, need2)
        Lassign[t] = best
        take1[:, t, :] = np.minimum(FL[:, t, :], best[0] - C1[:, t, :])

    # assign each flex edge a window: first take1[d] flex edges of node d -> w0
    flex_idx = np.where(flex)[0]
    fo = flex_idx[np.argsort(dst_r[flex_idx], kind="stable")]
    grp = dst_r[fo]
    starts = np.concatenate(([0], np.cumsum(np.bincount(grp, minlength=cfg.nrows))))[:-1]
    within = np.arange(len(fo)) - np.repeat(starts, np.bincount(grp, minlength=cfg.nrows))
    t1_of_edge = take1.reshape(-1)[
        (dst_core[fo] * cfg.tiles + dst_tile[fo]) * P + (dst_local[fo] % P)
    ]
    seg_src = np.empty(len(src_o), dtype=np.int64)
    seg_src[forced0] = 0
    seg_src[forced2] = 1
    seg_src[fo] = (within >= t1_of_edge).astype(np.int64)

    # Build schedule + idx arrays. The ucode trims a trailing run of NEGATIVE
    # indices from each call, which would drop real edges whose src row sits
    # below the window anchor. Fix per call by swapping a non-negative slot of
    # the same (node, window) group into the tail; where no such slot exists,
    # bump that (tile, window)'s scheduled depth by a pad layer and rebuild
    # (the new all-pad top layer is positive, so the tail is clean).
    import os as _os
    SUPER = int(_os.environ.get("SUPER", "1"))
    order_e = np.lexsort((src_r, seg_src, dst_r))  # by dst, then seg, then src
    s_r = src_r[order_e]
    d_r = dst_r[order_e]
    g_r = seg_src[order_e]
    floors = np.zeros_like(Lassign)

    for _attempt in range(6):
        Lmax = np.maximum(Lassign, floors)
        schedule = []  # (g, runs, off_w) with runs = [(tile, l_lo, l_hi), ...]
        off_w = 0
        for g in (0, 1):
            for st in range(0, cfg.tiles, SUPER):
                cur, cc = [], 0
                for t in range(st, min(st + SUPER, cfg.tiles)):
                    L = int(Lmax[t, g])
                    l = 0
                    while l < L:
                        take = min(cfg.lmax_call - cc, L - l)
                        cur.append((t, l, l + take))
                        cc += take
                        l += take
                        if cc == cfg.lmax_call:
                            schedule.append((g, cur, off_w))
                            off_w += cc * P // 16
                            cur, cc = [], 0
                if cc:
                    schedule.append((g, cur, off_w))
                    off_w += cc * P // 16
        total_w = off_w

        idx_arrays = []
        dirty = set()  # (t, g) whose call tail could not be cleaned by swaps
        for c in range(N_CORES):
            m = (d_r // cfg.percore) == c
            s_c, d_c, g_c = s_r[m], d_r[m] % cfg.percore, g_r[m]
            # slot index within (node, seg): position among equal (d_c, g_c)
            grp = d_c * 2 + g_c
            slot = np.arange(len(grp)) - np.repeat(
                np.concatenate(([0], np.cumsum(np.bincount(grp, minlength=cfg.percore * 2))))[:-1],
                np.bincount(grp, minlength=cfg.percore * 2),
            )
            tiles_c = d_c // P
            p_c = d_c % P
            rel = s_c - np.where(g_c == 0, cfg.seg_anchor[0], cfg.seg_anchor[1])
            out = np.empty((128, total_w), dtype=np.int16)
            chunk_base = {}
            for g, runs, ow in schedule:
                pad_rel = cfg.pad_row[g] - cfg.seg_anchor[g]
                cc = 0
                for (t, l_lo, l_hi) in runs:
                    for l in range(l_lo, l_hi):
                        chunk_base[(t, g, l)] = ow + cc * P // 16
                        cc += 1
                out[:, ow:ow + cc * P // 16] = pad_rel
            lmax_all = int(Lmax.max())
            cb = np.full((cfg.tiles, 2, lmax_all), -1, dtype=np.int64)
            for (t, g, l), wb in chunk_base.items():
                cb[t, g, l] = wb
            wb_arr = cb[tiles_c, g_c, slot]
            assert (wb_arr >= 0).all()
            w_pos = wb_arr + p_c // 16
            p_pos = p_c % 16
            vals = rel.astype(np.int16)
            for rep in range(8):
                out[p_pos + rep * 16, w_pos] = vals

            # clean call tails
            for (g, runs, ow) in schedule:
                C = sum(h - l for (_, l, h) in runs)
                pos = C * 128 - 1
                while pos >= 0:
                    if out[pos % 16, ow + pos // 16] >= 0:
                        break
                    ci, p = pos // 128, pos % 128
                    acc = 0
                    for (t, l_lo, l_hi) in runs:
                        if ci < acc + (l_hi - l_lo):
                            l = l_lo + (ci - acc)
                            break
                        acc += l_hi - l_lo
                    fixed = False
                    for l2 in range(l):
                        wb2 = chunk_base.get((t, g, l2))
                        if wb2 is None:
                            continue
                        v2 = out[p % 16, wb2 + p // 16]
                        if v2 >= 0:
                            v1 = out[p % 16, ow + pos // 16]
                            for rep in range(8):
                                out[p % 16 + rep * 16, wb2 + p // 16] = v1
                                out[p % 16 + rep * 16, ow + pos // 16] = v2
                            fixed = True
                            break
                    if fixed:
                        break
                    dirty.add((t, g))
                    pos -= 1

            idx_arrays.append(out)

        if not dirty:
            break
        for (t, g) in dirty:
            floors[t, g] = Lmax[t, g] + 1
    else:
        raise RuntimeError("call-tail cleaning did not converge")

    deg_full = deg + 1  # self loop
    return {
        "row_of": row_of,
        "core_of": core_of,
        "local_of": local_of,
        "deg_full": deg_full,
        "schedule": schedule,
        "total_w": total_w,
        "idx_arrays": idx_arrays,
        "Lmax": Lmax,
    }


def build_bass(cfg: Config, schedule, total_w, phases=(1, 2, 3)):
    H, FO, FI = cfg.hidden, cfg.f_out, cfg.f_in
    RP = cfg.row_pad
    PC, T = cfg.percore, cfg.tiles
    KC = FI // P  # W1 contraction chunks

    nc = bacc.Bacc(None, num_swdge_queues=4)
    xt = nc.dram_tensor("xt", [FI, PC], F32, kind="ExternalInput")
    w1 = nc.dram_tensor("w1", [FI, H], F32, kind="ExternalInput")
    w2 = nc.dram_tensor("w2", [H, FO], F32, kind="ExternalInput")
    b1t = nc.dram_tensor("b1t", [P, H], F32, kind="ExternalInput")
    b2t = nc.dram_tensor("b2t", [P, FO], F32, kind="ExternalInput")
    degt = nc.dram_tensor("degt", [PC], F32, kind="ExternalInput")
    maskt = nc.dram_tensor("maskt", [P, 2], F32, kind="ExternalInput")  # tile63/tile97
    idxs = nc.dram_tensor("idxs", [P, total_w], I16, kind="ExternalInput")
    out_d = nc.dram_tensor("out", [PC, FO], F32, kind="ExternalOutput")

    # piece-local staging tensors + windowed shared tables
    PSZ = [hi - lo for lo, hi in zip(cfg.piece_lo, cfg.piece_hi)]  # [4352,3840,4352]
    PT = [sz // P for sz in PSZ]          # piece sizes in tiles [34,30,34]
    PTLO = [lo // P for lo in cfg.piece_lo]  # first tile of piece [0,34,64]
    FLEXN = PSZ[1] * N_CORES              # 30720 rows in the flex band
    tl1 = [nc.dram_tensor(f"tab1_loc{p}", [PSZ[p], RP], F32) for p in range(3)]
    tl2 = [nc.dram_tensor(f"tab2_loc{p}", [PSZ[p], RP], F32) for p in range(3)]
    tab1_a = nc.dram_tensor("tab1_a", [65536, RP], F32, addr_space="Shared")
    tab1_b = nc.dram_tensor("tab1_b", [65536, RP], F32, addr_space="Shared")
    tab2_a = nc.dram_tensor("tab2_a", [65536, RP], F32, addr_space="Shared")
    tab2_b = nc.dram_tensor("tab2_b", [65536, RP], F32, addr_space="Shared")

    rg = [list(range(N_CORES))]

    def piece_of_tile(t):
        return 0 if t < PTLO[1] else (1 if t < PTLO[2] else 2)

    def emit_ag(tab_a, tab_b, loc, p):
        """AllGather piece p of a layer's table; after b also run the band copy."""
        base = cfg.piece_base[p]
        if p < 2:
            outs = [tab_a.ap()[base:base + PSZ[p] * N_CORES, :]]
        else:
            outs = [tab_b.ap()[FLEXN:, :]]
        nc.gpsimd.collective_compute(
            "AllGather", mybir.AluOpType.bypass,
            ins=[loc[p].ap()], outs=outs, replica_groups=rg,
        )
        if p == 1:
            # flex band has two addresses: copy tab_a[34816:65536] -> tab_b[0:30720]
            nc.sync.dma_start(out=tab_b.ap()[:FLEXN, :],
                              in_=tab_a.ap()[cfg.piece_base[1]:cfg.piece_base[2], :])

    with tile.TileContext(nc) as tc:
        with (
            tc.tile_pool(name="persist", bufs=1) as pp,
            tc.tile_pool(name="xs", bufs=2) as xs_pool,
            tc.tile_pool(name="work", bufs=int(__import__("os").environ.get("GBUFS", "8"))) as wp,
            tc.tile_pool(name="red", bufs=16) as rp,
            tc.tile_pool(name="psum", bufs=2, space="PSUM") as psp,
            tc.tile_pool(name="psum1", bufs=2, space="PSUM") as psp1,
        ):
            # ---- persistent small tensors ----
            w1_t = pp.tile([P, KC * H], F32)      # 4 chunks side by side
            nc.sync.dma_start(out=w1_t[:].rearrange("p (k h) -> p k h", k=KC),
                              in_=w1.ap().rearrange("(k p) h -> p k h", p=P))
            w2_t = pp.tile([H, FO], F32)
            nc.sync.dma_start(out=w2_t[:], in_=w2.ap())
            b1_t = pp.tile([P, H], F32)
            nc.sync.dma_start(out=b1_t[:], in_=b1t.ap())
            b2_t = pp.tile([P, FO], F32)
            nc.sync.dma_start(out=b2_t[:], in_=b2t.ap())
            mask_t = pp.tile([P, 2], F32)
            nc.sync.dma_start(out=mask_t[:], in_=maskt.ap())
            ident = pp.tile([P, P], F32)
            make_identity(nc, ident[:])
            deg_t = pp.tile([P, T], F32)
            nc.sync.dma_start(out=deg_t[:], in_=degt.ap().rearrange("(t p) -> p t", p=P))
            dinv_t = pp.tile([P, T], F32)
            nc.vector.reciprocal(out=dinv_t[:], in_=deg_t[:])
            nc.scalar.activation(out=dinv_t[:], in_=dinv_t[:],
                                 func=mybir.ActivationFunctionType.Sqrt)
            idx_all = pp.tile([P, total_w], I16)
            for lo in range(0, total_w, 8192):
                hi = min(total_w, lo + 8192)
                nc.sync.dma_start(out=idx_all[:, lo:hi], in_=idxs.ap()[:, lo:hi])
            acc1 = pp.tile([P, T * H], F32)     # per-tile partial-sum accumulators
            acc2 = pp.tile([P, T * H], F32)
            tab1_s = pp.tile([P, T * H], F32)   # resident own shard (table1)
            tab2_s = pp.tile([P, T * H], F32)

            # ---- phase 1: table1 = dinv * (X @ W1); AG each piece as it
            # completes so collectives overlap the remaining matmul work ----
            import os as _os
            xt_ap = xt.ap()
            for piece in range(3):
                plo, phi = cfg.piece_lo[piece], cfg.piece_hi[piece]
                BLK = (phi - plo) // 2
                for blk in range(plo, phi, BLK):
                    bw = BLK
                    chunks = []
                    for k in range(KC):
                        cte = xs_pool.tile([P, 2176], F32, tag=f"xt{k}")
                        nc.sync.dma_start(out=cte[:, :bw], in_=xt_ap[k * P:(k + 1) * P, blk:blk + bw])
                        chunks.append(cte)
                    for ti in range(bw // P):
                        t = blk // P + ti
                        ps = psp1.tile([P, H], F32, space="PSUM", tag="ps1")
                        for k in range(KC):
                            nc.tensor.matmul(
                                out=ps[:],
                                lhsT=chunks[k][:, ti * P:(ti + 1) * P],
                                rhs=w1_t[:, k * H:(k + 1) * H],
                                start=(k == 0), stop=(k == KC - 1),
                            )
                        # epilogue on ACT: table1_tile = dinv * ps
                        nc.scalar.activation(
                            out=tab1_s[:, t * H:(t + 1) * H], in_=ps[:],
                            func=mybir.ActivationFunctionType.Copy,
                            scale=dinv_t[:, t:t + 1],
                        )
                        stage = wp.tile([P, RP], F32, tag="stage")
                        if t < 8:
                            nc.vector.memset(stage[:], 0.0)
                        nc.vector.tensor_copy(out=stage[:, :H], in_=tab1_s[:, t * H:(t + 1) * H])
                        nc.sync.dma_start(
                            out=tl1[piece].ap()[(t - PTLO[piece]) * P:(t - PTLO[piece] + 1) * P, :],
                            in_=stage[:],
                        )
                emit_ag(tab1_a, tab1_b, tl1, piece)

            chunks_of = {}   # t -> number of chunks expected
            for g, runs, ow in schedule:
                for (t, l_lo, l_hi) in runs:
                    chunks_of[t] = chunks_of.get(t, 0) + (l_hi - l_lo)

            def agg_layer(tab_a, tab_b, layer, epilogue):
                """Packed calls; per-tile strided reduces accumulated into a
                persistent acc tile (g-major call order holds partials across
                the whole window-0 stream). epilogue(t, u) fires once a tile's
                chunks are all reduced. Window 0 calls read tab_a (ready after
                AG a+b), window 1 calls read tab_b."""
                own = tab1_s if layer == 1 else tab2_s
                acc = acc1 if layer == 1 else acc2
                done = {t: 0 for t in range(T)}

                def finish(t):
                    u = rp.tile([P, H], F32, tag=f"u{layer}")
                    nc.vector.tensor_add(out=u[:], in0=acc[:, t * H:(t + 1) * H],
                                         in1=own[:, t * H:(t + 1) * H])
                    epilogue(t, u)

                qload = [0, 0, 0, 0]
                for call_i, (g, runs, ow) in enumerate(schedule):
                    C = sum(l_hi - l_lo for (_, l_lo, l_hi) in runs)
                    ni = C * P
                    gt = wp.tile([P, C * H], F32, tag="g")
                    tab = tab_a if g == 0 else tab_b
                    q = min(range(4), key=lambda i: qload[i])
                    qload[q] += ni
                    DMA_GATHER_RAW(
                        nc.gpsimd,
                        gt[:].rearrange("p (c h) -> p c h", c=C),
                        tab.ap()[32768:, :H],
                        idx_all[:, ow:ow + C * P // 16],
                        ni, ni, H, elem_step=RP,
                        queue_num=q,
                        single_packet=False,
                    )
                    off = 0
                    for (t, l_lo, l_hi) in runs:
                        n = l_hi - l_lo
                        if done[t] == 0:
                            # first chunk group of this tile: reduce into acc
                            nc.vector.tensor_reduce(
                                out=acc[:, t * H:(t + 1) * H],
                                in_=gt[:, off * H:(off + n) * H].rearrange("p (l h) -> p h l", h=H),
                                op=mybir.AluOpType.add, axis=mybir.AxisListType.X,
                            )
                        else:
                            red = rp.tile([P, H], F32, tag=f"red{layer}")
                            nc.vector.tensor_reduce(
                                out=red[:],
                                in_=gt[:, off * H:(off + n) * H].rearrange("p (l h) -> p h l", h=H),
                                op=mybir.AluOpType.add, axis=mybir.AxisListType.X,
                            )
                            nc.vector.tensor_add(
                                out=acc[:, t * H:(t + 1) * H],
                                in0=acc[:, t * H:(t + 1) * H], in1=red[:],
                            )
                        done[t] += n
                        off += n
                        if done[t] == chunks_of[t]:
                            finish(t)

            # ---- phase 2: layer-1 aggregation + epilogue -> table2;
            # AG2 pieces fire as the last tile of each piece finishes ----
            def epi1(t, u):
                # v = u*dinv + b1  -> table2_tile = dinv * relu(v)
                nc.vector.tensor_scalar_mul(out=u[:], in0=u[:], scalar1=dinv_t[:, t:t + 1])
                nc.vector.tensor_add(out=u[:], in0=u[:], in1=b1_t[:])
                nc.scalar.activation(
                    out=tab2_s[:, t * H:(t + 1) * H], in_=u[:],
                    func=mybir.ActivationFunctionType.Relu,
                    scale=dinv_t[:, t:t + 1],
                )
                if t == PTLO[2] - 1 or t == T - 1:
                    # zero the reserved/fake rows (locals 8191 and >=12502)
                    col = 0 if t == PTLO[2] - 1 else 1
                    nc.vector.tensor_scalar_mul(
                        out=tab2_s[:, t * H:(t + 1) * H],
                        in0=tab2_s[:, t * H:(t + 1) * H], scalar1=mask_t[:, col:col + 1],
                    )
                piece = piece_of_tile(t)
                stage = wp.tile([P, RP], F32, tag="stage2")
                if t < 8:
                    nc.vector.memset(stage[:], 0.0)
                nc.vector.tensor_copy(out=stage[:, :H], in_=tab2_s[:, t * H:(t + 1) * H])
                nc.sync.dma_start(
                    out=tl2[piece].ap()[(t - PTLO[piece]) * P:(t - PTLO[piece] + 1) * P, :],
                    in_=stage[:],
                )
                if t == PTLO[1] - 1:
                    emit_ag(tab2_a, tab2_b, tl2, 0)
                elif t == PTLO[2] - 1:
                    emit_ag(tab2_a, tab2_b, tl2, 1)
                elif t == T - 1:
                    emit_ag(tab2_a, tab2_b, tl2, 2)

            agg_layer(tab1_a, tab1_b, 1, epi1)

            # ---- phase 3: layer-2 aggregation + W2 + b2 ----
            def epi2(t, u):
                # transpose u -> [H, P], matmul with W2, scale by dinv, + b2
                ps_t = psp.tile([P, P], F32, space="PSUM", tag="pst")
                nc.tensor.transpose(out=ps_t[:H, :], in_=u[:], identity=ident[:])
                uT = rp.tile([H, P], F32, tag="uT")
                nc.vector.tensor_copy(out=uT[:], in_=ps_t[:H, :])
                ps_o = psp.tile([P, FO], F32, space="PSUM", tag="pso")
                nc.tensor.matmul(out=ps_o[:], lhsT=uT[:], rhs=w2_t[:], start=True, stop=True)
                ot = rp.tile([P, FO], F32, tag="ot")
                nc.vector.tensor_scalar_mul(out=ot[:], in0=ps_o[:], scalar1=dinv_t[:, t:t + 1])
                nc.vector.tensor_add(out=ot[:], in0=ot[:], in1=b2_t[:])
                nc.sync.dma_start(out=out_d.ap()[t * P:(t + 1) * P, :], in_=ot[:])

            agg_layer(tab2_a, tab2_b, 2, epi2)

    nc.finalize()
    return nc


_CACHE = {}


def _get_compiled(cfg_key, cfg, prep):
    if cfg_key not in _CACHE:
        nc = build_bass(cfg, prep["schedule"], prep["total_w"])
        _CACHE[cfg_key] = nc
    return _CACHE[cfg_key]


def build_in_maps(cfg, prep, x, W1, b1, W2, b2):
    x = np.asarray(x, dtype=np.float32)
    row_of = prep["row_of"]
    deg_full = prep["deg_full"].astype(np.float32)
    x_rows = np.zeros((cfg.nrows, cfg.f_in), dtype=np.float32)
    x_rows[row_of] = x
    deg_rows = np.ones(cfg.nrows, dtype=np.float32)
    deg_rows[row_of] = deg_full
    b1t = np.tile(np.asarray(b1, np.float32)[None, :], (P, 1))
    b2t = np.tile(np.asarray(b2, np.float32)[None, :], (P, 1))
    # occupancy-based masks for the two tiles containing fake rows:
    # col 0 -> tile 63 (reserved local 8191), col 1 -> last tile
    occ = np.zeros(cfg.percore, dtype=np.float32)
    n_pc = cfg.n_nodes // N_CORES
    loc_slots = np.concatenate(
        [np.arange(cfg.reserved_local), np.arange(cfg.reserved_local + 1, cfg.percore)]
    )[:n_pc]
    occ[loc_slots] = 1.0
    t63 = cfg.piece_hi[1] // P - 1
    mask = np.zeros((P, 2), dtype=np.float32)
    mask[:, 0] = occ[t63 * P:(t63 + 1) * P]
    mask[:, 1] = occ[(cfg.tiles - 1) * P:cfg.tiles * P]
    in_maps = []
    for c in range(N_CORES):
        xs = x_rows[c * cfg.percore:(c + 1) * cfg.percore]
        in_maps.append({
            "xt": np.ascontiguousarray(xs.T),
            "w1": np.asarray(W1, np.float32), "w2": np.asarray(W2, np.float32),
            "b1t": b1t, "b2t": b2t,
            "degt": deg_rows[c * cfg.percore:(c + 1) * cfg.percore],
            "maskt": mask,
            "idxs": prep["idx_arrays"][c],
        })
    return in_maps


def run(x, edge_index, W1, b1, W2, b2, cfg: Config, prep=None, nc=None, time_iters=0):
    x = np.asarray(x, dtype=np.float32)
    W1 = np.asarray(W1, dtype=np.float32)
    b1 = np.asarray(b1, dtype=np.float32)
    W2 = np.asarray(W2, dtype=np.float32)
    b2 = np.asarray(b2, dtype=np.float32)
    if prep is None:
        prep = host_prep(x, edge_index, cfg)
    if nc is None:
        nc = _get_compiled(("main", cfg.n_nodes, cfg.percore), cfg, prep)

    in_maps = build_in_maps(cfg, prep, x, W1, b1, W2, b2)
    row_of = prep["row_of"]

    import time as _time
    res = run_bass_kernel_spmd(nc, in_maps, core_ids=list(range(N_CORES)))
    timing = None
    if time_iters:
        t0 = _time.time()
        for _ in range(time_iters):
            res = run_bass_kernel_spmd(nc, in_maps, core_ids=list(range(N_CORES)))
        t1 = _time.time()
        timing = (t1 - t0) / time_iters
    out_rows = np.concatenate([res.results[c]["out"] for c in range(N_CORES)], axis=0)
    out = out_rows[row_of]
    return (out, timing) if time_iters else out


def kernel(x, edge_index, W1, b1, W2, b2):
    cfg = Config(100000, 512, 16, 40, percore=12544)
    return run(x, edge_index, W1, b1, W2, b2, cfg)

